# revision 7
# baseline (speedup 1.0000x reference)
"""Trainium2 Bass kernel for nn_Attention_83330955478086.

Full attention layer: QKV projections + (degenerate) rotary + causal softmax
attention + output projection.  x:(1,2048,4096), 32 heads x 128 head_dim.

Sharding: tensor-parallel over heads. Each of the 8 cores computes 4 heads
(d-shard of 512) of Q/K/V, runs attention for those heads, then the cores
AllGather the attention outputs and each computes a 512-column slice of the
final output projection.  Host concatenates the slices.

Key layout trick: everything on-chip is kept "transposed" ([feature, seq]) so
no on-device transposes are needed anywhere:
  - host feeds x^T, wq^T, wk^T, wv^T, wo^T (marshalling)
  - Q/K projections emit Q^T/K^T tiles [head_dim, seq] directly
  - scores are computed transposed: scoresT[k,q] = sum_hd K^T[hd,k]*Q^T[hd,q]
  - softmax: exp on ACT; k-sums via ones-matmul; bcast-reciprocal normalize
  - PV uses V in natural [seq, d] layout as the stationary operand and emits
    attn^T [hd, q]; AllGather concatenates attn^T on the feature axis
  - output projection emits out^T [512, 2048]; host transposes back.

Rotary here degenerates to an elementwise scale (the reference's pair-swap is
the identity):  out[2j] = q[2j]*(c_j - s_j), out[2j+1] = q[2j+1]*(c_j + s_j).
We permute the wq/wk columns per head (even hd first, odd hd second, on the
host) so the device multiplies by a [cos-sin; cos+sin] stacked tile without
any interleaved-partition access.  The permutation cancels in the q.k dot.

Matmuls run as float32r (single-pass fp32, ~1.5e-4 rel err, full PE rate).
"""
import math
import os

import numpy as np

import concourse.bacc as bacc
import concourse.tile as tile
from concourse import mybir
from concourse.bass_utils import run_bass_kernel_spmd

N_CORES = 8
S = 2048
D = 4096
H = 32
HD = 128
DSH = D // N_CORES  # 512 per-core d shard
HL = DSH // HD  # 4 heads per core
KT = D // 128  # 32 contraction tiles for the projections
SC = S // 512  # 4 seq chunks of 512
ST = S // 128  # 16 seq tiles of 128

F32 = mybir.dt.float32
F32R = mybir.dt.float32r

# mask-block classes
B_SKIP = 0  # fully masked (mask < -1e4): exp underflows to exactly 0 -> skip
B_ZERO = 1  # mask identically 0: skip the add
B_ADD = 2  # mixed: stream the mask tile and add


def _build(cls_grid):
    nc = bacc.Bacc(
        "TRN2", target_bir_lowering=False, debug=False, num_devices=N_CORES
    )

    xT = nc.dram_tensor("xT", [D, S], F32R, kind="ExternalInput")
    wqT = nc.dram_tensor("wqT", [D, DSH], F32R, kind="ExternalInput")
    wkT = nc.dram_tensor("wkT", [D, DSH], F32R, kind="ExternalInput")
    wvT = nc.dram_tensor("wvT", [D, DSH], F32R, kind="ExternalInput")
    woT = nc.dram_tensor("woT", [D, DSH], F32R, kind="ExternalInput")
    freqT = nc.dram_tensor("freqT", [128, S], F32, kind="ExternalInput")
    ones_in = nc.dram_tensor("ones_in", [128, 1], F32R, kind="ExternalInput")
    maskT = nc.dram_tensor("maskT", [S, S], F32, kind="ExternalInput")
    outT = nc.dram_tensor("outT", [DSH, S], F32, kind="ExternalOutput")

    qT_d = nc.dram_tensor("qT_d", [DSH, S], F32R)
    kT_d = nc.dram_tensor("kT_d", [DSH, S], F32R)
    attnT_d = nc.dram_tensor("attnT_d", [DSH, S], F32R)
    attnT_full = nc.dram_tensor("attnT_full", [D, S], F32R, addr_space="Shared")

    with tile.TileContext(nc) as tc:
        # ---------- pass A: Q^T and K^T projections (+ rotary on evac) ------
        with (
            tc.tile_pool(name="pa_w", bufs=1) as pa_w,
            tc.tile_pool(name="pa_x", bufs=6) as pa_x,
            tc.tile_pool(name="pa_ps", bufs=1, space="PSUM") as pa_ps,
            tc.tile_pool(name="pa_ev", bufs=4) as pa_ev,
        ):
            wq_sb = pa_w.tile([128, KT * DSH], F32R)
            nc.sync.dma_start(
                wq_sb[:].rearrange("p (t d) -> p t d", d=DSH),
                wqT.ap().rearrange("(t p) d -> p t d", p=128),
            )
            wk_sb = pa_w.tile([128, KT * DSH], F32R)
            nc.scalar.dma_start(
                wk_sb[:].rearrange("p (t d) -> p t d", d=DSH),
                wkT.ap().rearrange("(t p) d -> p t d", p=128),
            )
            # rotary multipliers from freqs: rows 0-63 cos, 64-127 sin.
            # DVE tensor_tensor needs equal base partitions, so compute the
            # halves in [64, S] tiles and assemble gk via SBUF->SBUF DMA.
            fcos = pa_w.tile([64, S], F32)
            nc.sync.dma_start(fcos[:], freqT.ap()[0:64, :])
            fsin = pa_w.tile([64, S], F32)
            nc.sync.dma_start(fsin[:], freqT.ap()[64:128, :])
            cms = pa_w.tile([64, S], F32)
            nc.vector.tensor_sub(cms[:], fcos[:], fsin[:])
            cps = pa_w.tile([64, S], F32)
            nc.vector.tensor_add(cps[:], fcos[:], fsin[:])
            gk = pa_w.tile([128, S], F32)
            nc.sync.dma_start(gk[0:64, :], cms[:])
            nc.sync.dma_start(gk[64:128, :], cps[:])
            gq = pa_w.tile([128, S], F32)
            nc.scalar.mul(gq[:], gk[:], 1.0 / math.sqrt(HD))

            for sc in range(SC):
                psq = [
                    pa_ps.tile([128, 512], F32, name=f"psq{i}") for i in range(HL)
                ]
                psk = [
                    pa_ps.tile([128, 512], F32, name=f"psk{i}") for i in range(HL)
                ]
                for kc in range(KT):
                    xt = pa_x.tile([128, 512], F32R)
                    eng = nc.sync if kc % 2 == 0 else nc.scalar
                    eng.dma_start(
                        xt[:],
                        xT.ap()[kc * 128 : (kc + 1) * 128, sc * 512 : (sc + 1) * 512],
                    )
                    for dt in range(HL):
                        nc.tensor.matmul(
                            psq[dt][:],
                            (wq_sb[:, kc * DSH + dt * 128 : kc * DSH + (dt + 1) * 128]),
                            (xt[:]),
                            start=(kc == 0),
                            stop=(kc == KT - 1),
                        )
                    for dt in range(HL):
                        nc.tensor.matmul(
                            psk[dt][:],
                            (wk_sb[:, kc * DSH + dt * 128 : kc * DSH + (dt + 1) * 128]),
                            (xt[:]),
                            start=(kc == 0),
                            stop=(kc == KT - 1),
                        )
                for dt in range(HL):
                    qev = pa_ev.tile([128, 512], F32R, name="qev")
                    nc.vector.tensor_mul(
                        qev[:], psq[dt][:], gq[:, sc * 512 : (sc + 1) * 512]
                    )
                    nc.sync.dma_start(
                        qT_d.ap()[
                            dt * 128 : (dt + 1) * 128, sc * 512 : (sc + 1) * 512
                        ],
                        qev[:],
                    )
                    kev = pa_ev.tile([128, 512], F32R, name="kev")
                    nc.vector.tensor_mul(
                        kev[:], psk[dt][:], gk[:, sc * 512 : (sc + 1) * 512]
                    )
                    nc.scalar.dma_start(
                        kT_d.ap()[
                            dt * 128 : (dt + 1) * 128, sc * 512 : (sc + 1) * 512
                        ],
                        kev[:],
                    )

        # ---------- pass B: V projection (natural [s, d] layout) ------------
        with tc.tile_pool(name="pv_keep", bufs=1) as pv_keep:
            v_sb = pv_keep.tile([128, ST * DSH], F32R)  # 4 MB, persists to ph.2
            with (
                tc.tile_pool(name="pb_w", bufs=1) as pb_w,
                tc.tile_pool(name="pb_x", bufs=4) as pb_x,
                tc.tile_pool(name="pb_ps", bufs=1, space="PSUM") as pb_ps,
            ):
                wv_sb = pb_w.tile([128, KT * DSH], F32R)
                nc.sync.dma_start(
                    wv_sb[:].rearrange("p (t d) -> p t d", d=DSH),
                    wvT.ap().rearrange("(t p) d -> p t d", p=128),
                )
                for sh in range(2):
                    psv = [
                        pb_ps.tile([128, 512], F32, name=f"psv{i}") for i in range(8)
                    ]
                    for kc in range(KT):
                        xt2 = pb_x.tile([128, 1024], F32R)
                        eng = nc.sync if kc % 2 == 0 else nc.scalar
                        eng.dma_start(
                            xt2[:],
                            xT.ap()[
                                kc * 128 : (kc + 1) * 128,
                                sh * 1024 : (sh + 1) * 1024,
                            ],
                        )
                        for st in range(8):
                            nc.tensor.matmul(
                                psv[st][:],
                                (xt2[:, st * 128 : (st + 1) * 128]),
                                (wv_sb[:, kc * DSH : (kc + 1) * DSH]),
                                start=(kc == 0),
                                stop=(kc == KT - 1),
                            )
                    for st in range(8):
                        gt = sh * 8 + st  # global s-tile 0..15
                        nc.vector.tensor_copy(
                            v_sb[:, gt * DSH : (gt + 1) * DSH], psv[st][:]
                        )

            # ---------- phase 2: attention per head -------------------------
            with (
                tc.tile_pool(name="p2_qk", bufs=2) as p2_qk,
                tc.tile_pool(name="p2_m", bufs=4) as p2_m,
                tc.tile_pool(name="p2_ex", bufs=6) as p2_ex,
                tc.tile_pool(name="p2_sm", bufs=4) as p2_sm,
                tc.tile_pool(name="p2_at", bufs=4) as p2_at,
                tc.tile_pool(name="p2_sc", bufs=4, space="PSUM") as p2_sc,
                tc.tile_pool(name="p2_ap", bufs=2, space="PSUM") as p2_ap,
                tc.tile_pool(name="p2_sp", bufs=2, space="PSUM") as p2_sp,
                tc.tile_pool(name="p2_one", bufs=1) as p2_one,
            ):
                ones_t = p2_one.tile([128, 1], F32R)
                nc.sync.dma_start(ones_t[:], ones_in.ap())
                for h in range(HL):
                    kt_sb = p2_qk.tile([128, S], F32R, name="kt_sb")
                    nc.sync.dma_start(kt_sb[:], kT_d.ap()[h * 128 : (h + 1) * 128, :])
                    qt_sb = p2_qk.tile([128, S], F32R, name="qt_sb")
                    nc.scalar.dma_start(
                        qt_sb[:], qT_d.ap()[h * 128 : (h + 1) * 128, :]
                    )
                    for qc in range(SC):
                        live = [
                            kt for kt in range(ST) if cls_grid[kt][qc] != B_SKIP
                        ]
                        att_ps = p2_ap.tile([128, 512], F32, name="att_ps")
                        sum_ps = p2_sp.tile([1, 512], F32, name="sum_ps")
                        for i, kt in enumerate(live):
                            first = i == 0
                            last = i == len(live) - 1
                            sc_ps = p2_sc.tile([128, 512], F32, name="sc_ps")
                            nc.tensor.matmul(
                                sc_ps[:],
                                (kt_sb[:, kt * 128 : (kt + 1) * 128]),
                                (qt_sb[:, qc * 512 : (qc + 1) * 512]),
                                start=True,
                                stop=True,
                            )
                            if cls_grid[kt][qc] == B_ADD:
                                mk = p2_m.tile([128, 512], F32)
                                nc.scalar.dma_start(
                                    mk[:],
                                    maskT.ap()[
                                        kt * 128 : (kt + 1) * 128,
                                        qc * 512 : (qc + 1) * 512,
                                    ],
                                )
                                nc.vector.tensor_add(sc_ps[:], sc_ps[:], mk[:])
                            ex = p2_ex.tile([128, 512], F32R)
                            nc.scalar.activation(
                                ex[:], sc_ps[:], mybir.ActivationFunctionType.Exp
                            )
                            nc.tensor.matmul(
                                att_ps[:],
                                (v_sb[:, kt * DSH + h * 128 : kt * DSH + (h + 1) * 128]),
                                (ex[:]),
                                start=first,
                                stop=last,
                            )
                            nc.tensor.matmul(
                                sum_ps[:],
                                (ones_t[:]),
                                (ex[:]),
                                start=first,
                                stop=last,
                            )
                        rec = p2_sm.tile([1, 512], F32, name="rec")
                        nc.vector.reciprocal(rec[:], sum_ps[0:1, :])
                        rb = p2_sm.tile([128, 512], F32, name="rb")
                        nc.gpsimd.partition_broadcast(rb[:], rec[0:1, :])
                        at = p2_at.tile([128, 512], F32R)
                        nc.vector.tensor_mul(at[:], att_ps[:], rb[:])
                        nc.sync.dma_start(
                            attnT_d.ap()[
                                h * 128 : (h + 1) * 128, qc * 512 : (qc + 1) * 512
                            ],
                            at[:],
                        )

        # ---------- AllGather attn^T across cores ---------------------------
        nc.gpsimd.collective_compute(
            "AllGather",
            mybir.AluOpType.bypass,
            ins=[attnT_d.ap()],
            outs=[attnT_full.ap()],
            replica_groups=[list(range(N_CORES))],
        )

        # ---------- phase 3: output projection slice ------------------------
        with (
            tc.tile_pool(name="p3_w", bufs=1) as p3_w,
            tc.tile_pool(name="p3_a", bufs=6) as p3_a,
            tc.tile_pool(name="p3_ps", bufs=2, space="PSUM") as p3_ps,
            tc.tile_pool(name="p3_ev", bufs=4) as p3_ev,
        ):
            wo_sb = p3_w.tile([128, KT * DSH], F32R)
            nc.sync.dma_start(
                wo_sb[:].rearrange("p (t d) -> p t d", d=DSH),
                woT.ap().rearrange("(t p) d -> p t d", p=128),
            )
            for sc in range(SC):
                pso = [
                    p3_ps.tile([128, 512], F32, name=f"pso{i}") for i in range(HL)
                ]
                for dc in range(KT):
                    at_t = p3_a.tile([128, 512], F32R)
                    eng = nc.sync if dc % 2 == 0 else nc.scalar
                    eng.dma_start(
                        at_t[:],
                        attnT_full.ap()[
                            dc * 128 : (dc + 1) * 128, sc * 512 : (sc + 1) * 512
                        ],
                    )
                    for jt in range(HL):
                        nc.tensor.matmul(
                            pso[jt][:],
                            (wo_sb[:, dc * DSH + jt * 128 : dc * DSH + (jt + 1) * 128]),
                            (at_t[:]),
                            start=(dc == 0),
                            stop=(dc == KT - 1),
                        )
                for jt in range(HL):
                    oev = p3_ev.tile([128, 512], F32)
                    nc.vector.tensor_copy(oev[:], pso[jt][:])
                    nc.sync.dma_start(
                        outT.ap()[
                            jt * 128 : (jt + 1) * 128, sc * 512 : (sc + 1) * 512
                        ],
                        oev[:],
                    )

    nc.compile()
    return nc


def _install_trace_hooks():
    """Install the NTFF profile hook (missing antenv.axon_hooks stub) and
    neutralize the artifact upload so trace=True works in this container."""
    import sys
    import types

    from concourse import bass_utils as _bu

    _bu.upload_artifacts = lambda tmpdir: f"file://{tmpdir}"
    if "antenv.axon_hooks" in sys.modules:
        return
    import antenv

    mod = types.ModuleType("antenv.axon_hooks")
    _h = [None]
    mod.set_axon_ntff_profile_hook = lambda hk: _h.__setitem__(0, hk)
    mod.get_axon_ntff_profile_hook = lambda: _h[0]
    sys.modules["antenv.axon_hooks"] = mod
    antenv.axon_hooks = mod
    from trn_agent_boot.trn_boot import _ntff_profile_via_ctypes

    mod.set_axon_ntff_profile_hook(
        _ntff_profile_via_ctypes("/opt/axon/libaxon_pjrt.so")
    )


_CACHE = {}


def _get_program(cls_grid):
    key = tuple(map(tuple, cls_grid))
    if key not in _CACHE:
        _CACHE[key] = _build(cls_grid)
    return _CACHE[key]


def _classify_mask(maskT_np):
    """Classify each [128k, 512q] block of the transposed mask."""
    grid = []
    for kt in range(ST):
        row = []
        for qc in range(SC):
            blk = maskT_np[kt * 128 : (kt + 1) * 128, qc * 512 : (qc + 1) * 512]
            if np.all(blk < -1e4):
                row.append(B_SKIP)
            elif np.all(blk == 0.0):
                row.append(B_ZERO)
            else:
                row.append(B_ADD)
        grid.append(row)
    return grid


_ONES = np.ones((128, 1), dtype=np.float32)

# within-head permutation: even head_dim indices first, then odd
_PERM = np.empty(DSH, dtype=np.int64)
for _hl in range(HL):
    for _j in range(64):
        _PERM[_hl * 128 + _j] = _hl * 128 + 2 * _j
        _PERM[_hl * 128 + 64 + _j] = _hl * 128 + 2 * _j + 1


def kernel(x, start_pos, freqs, mask, wq, wk, wv, wo):
    x = np.asarray(x, dtype=np.float32)
    freqs = np.asarray(freqs, dtype=np.float32)
    mask = np.asarray(mask, dtype=np.float32)
    wq = np.asarray(wq, dtype=np.float32)
    wk = np.asarray(wk, dtype=np.float32)
    wv = np.asarray(wv, dtype=np.float32)
    wo = np.asarray(wo, dtype=np.float32)

    xs = x.reshape(S, D)
    xT = np.ascontiguousarray(xs.T)
    freqT = np.ascontiguousarray(
        np.concatenate([freqs[:, :, 0].T, freqs[:, :, 1].T], axis=0)
    )  # [128, S]: rows 0-63 cos_j(s), 64-127 sin_j(s)
    maskT_np = np.ascontiguousarray(mask.reshape(S, S).T)
    cls_grid = _classify_mask(maskT_np)
    nc = _get_program(cls_grid)

    in_maps = []
    for c in range(N_CORES):
        rows = slice(c * DSH, (c + 1) * DSH)
        wq_c = wq[rows][_PERM]  # permute within-head rows (even hd, odd hd)
        wk_c = wk[rows][_PERM]
        in_maps.append(
            {
                "xT": xT,
                "wqT": np.ascontiguousarray(wq_c.T),
                "wkT": np.ascontiguousarray(wk_c.T),
                "wvT": np.ascontiguousarray(wv[rows].T),
                "woT": np.ascontiguousarray(wo[rows].T),
                "freqT": freqT,
                "ones_in": _ONES,
                "maskT": maskT_np,
            }
        )

    trace = os.environ.get("ATTN_TRACE") == "1"
    if trace:
        try:
            _install_trace_hooks()
        except Exception:
            pass

    res = run_bass_kernel_spmd(
        nc,
        in_maps,
        list(range(N_CORES)),
        trace=trace,
        trace_cores=[0] if trace else None,
    )
    if trace:
        kernel.last_exec_time_ns = res.exec_time_ns
        kernel.last_results = res

    out = np.empty((S, D), dtype=np.float32)
    for c in range(N_CORES):
        out[:, c * DSH : (c + 1) * DSH] = res.results[c]["outT"].T
    return out[None]


# revision 20
# speedup vs baseline: 1.1288x; 1.1288x over previous
"""Trainium2 Bass kernel for nn_Attention_83330955478086.

Full attention layer: QKV projections + (degenerate) rotary + causal softmax
attention + output projection.  x:(1,2048,4096), 32 heads x 128 head_dim.

Sharding: tensor-parallel over heads. Each of the 8 cores computes 4 heads
(d-shard of 512) of Q/K/V, runs attention for those heads, then the cores
AllGather the attention outputs (two 1024-seq chunks, pipelined against the
remaining attention work) and each computes a 512-column slice of the final
output projection.  Host concatenates the slices.

Layout: everything on-chip is "transposed" ([feature, seq]) so no on-device
transposes are needed anywhere:
  - host feeds x^T, wq^T, wk^T, wv^T, wo^T (marshalling)
  - Q/K projections emit Q^T/K^T tiles [head_dim, seq] directly
  - scores are computed transposed: scoresT[k,q] = sum_hd K^T[hd,k]*Q^T[hd,q]
  - softmax: exp on ACT; k-sums via ones-matmul; bcast-reciprocal normalize
  - PV uses V in natural [seq, d] layout as the stationary operand and emits
    attn^T [hd, q]; AllGather concatenates attn^T on the feature axis
  - output projection emits out^T [512, 2048]; host transposes back.

Rotary degenerates to an elementwise scale (the reference's pair-swap is the
identity): out[2j] = q[2j]*(c_j - s_j), out[2j+1] = q[2j+1]*(c_j + s_j).
We permute the wq/wk columns per head (even hd first, odd hd second, on the
host) so the device multiplies by a [cos-sin; cos+sin] stacked tile without
interleaved-partition access.  The permutation cancels in the q.k contraction.

Matmuls run as float32r (single-pass fp32, ~2e-4 rel err, full PE rate).
"""
import math
import os

import numpy as np

import concourse.bacc as bacc
import concourse.tile as tile
from concourse.tile import add_dep_helper
from concourse import mybir
from concourse.bass_utils import run_bass_kernel_spmd

N_CORES = 8
S = 2048
D = 4096
H = 32
HD = 128
DSH = D // N_CORES  # 512 per-core d shard
HL = DSH // HD  # 4 heads per core
KT = D // 128  # 32 contraction tiles for the projections
SC = S // 512  # 4 seq chunks of 512
ST = S // 128  # 16 seq tiles of 128

F32 = mybir.dt.float32
F32R = mybir.dt.float32r

# mask-block classes
B_SKIP = 0  # fully masked (mask < -1e4): exp underflows to exactly 0 -> skip
B_ZERO = 1  # mask identically 0: skip the add
B_ADD = 2  # mixed: stream the mask tile and add


def _w_load(nc, sb_tile, dram, kt0, kt1, n_chunks, engines):
    """Load kc tiles [kt0, kt1) of a [D, DSH] weight into `sb_tile` (kc-major
    [128, (kt1-kt0)*DSH]) in chunks so consumers start after ~1/n of the load."""
    ktn = kt1 - kt0
    kt_per = ktn // n_chunks
    for g in range(n_chunks):
        eng = getattr(nc, engines[g % len(engines)])
        eng.dma_start(
            sb_tile[:, g * kt_per * DSH : (g + 1) * kt_per * DSH].rearrange(
                "p (t d) -> p t d", d=DSH
            ),
            dram.ap()[
                (kt0 + g * kt_per) * 128 : (kt0 + (g + 1) * kt_per) * 128, :
            ].rearrange("(t p) d -> p t d", p=128),
        )


def _build(cls_grid):
    nc = bacc.Bacc(
        "TRN2", target_bir_lowering=False, debug=False, num_devices=N_CORES
    )

    xT = nc.dram_tensor("xT", [D, S], F32R, kind="ExternalInput")
    wqT = nc.dram_tensor("wqT", [D, DSH], F32R, kind="ExternalInput")
    wkT = nc.dram_tensor("wkT", [D, DSH], F32R, kind="ExternalInput")
    wvT = nc.dram_tensor("wvT", [D, DSH], F32R, kind="ExternalInput")
    woT = nc.dram_tensor("woT", [D, DSH], F32R, kind="ExternalInput")
    freqT = nc.dram_tensor("freqT", [128, S], F32, kind="ExternalInput")
    ones_in = nc.dram_tensor("ones_in", [128, 1], F32R, kind="ExternalInput")
    maskT = nc.dram_tensor("maskT", [S, S], F32, kind="ExternalInput")
    outT = nc.dram_tensor("outT", [DSH, S], F32, kind="ExternalOutput")

    qT_d = nc.dram_tensor("qT_d", [DSH, S], F32R)
    kT_d = nc.dram_tensor("kT_d", [DSH, S], F32R)
    attn_sc = [nc.dram_tensor(f"attn_sc{i}", [DSH, 1024], F32R) for i in range(2)]
    attn_full = [
        nc.dram_tensor(f"attn_full{i}", [D, 1024], F32R, addr_space="Shared")
        for i in range(2)
    ]

    with tile.TileContext(nc) as tc, tc.tile_pool(
        name="pv_keep", bufs=1
    ) as pv_keep, tc.tile_pool(name="p2_k", bufs=1) as p2_k:
        v_sb = pv_keep.tile([128, ST * DSH], F32R)  # V, persists to phase 2
        kres = [
            p2_k.tile([128, S], F32R, name=f"kres{h}") for h in range(HL)
        ]

        # ---- passes A1/A2/B: Q^T, K^T (transposed) and V projections ----
        with (
            tc.tile_pool(name="px", bufs=4) as px,
            tc.tile_pool(name="pg", bufs=1) as pg,
            tc.tile_pool(name="pev", bufs=3) as pev,
        ):
            # rotary multipliers: gk rows 0-63 = cos-sin, 64-127 = cos+sin;
            # gq = gk/sqrt(HD).  DVE tensor_tensor needs equal base
            # partitions, so compute in [64, *] tiles, assemble via DMA.
            gk = pg.tile([128, S], F32)
            gq = pg.tile([128, S], F32)
            with tc.tile_pool(name="pa_f", bufs=1) as pa_f:
                HS = S // 2
                for fh in range(2):
                    fcos = pa_f.tile([64, HS], F32, name="fcos")
                    nc.sync.dma_start(
                        fcos[:], freqT.ap()[0:64, fh * HS : (fh + 1) * HS]
                    )
                    fsin = pa_f.tile([64, HS], F32, name="fsin")
                    nc.sync.dma_start(
                        fsin[:], freqT.ap()[64:128, fh * HS : (fh + 1) * HS]
                    )
                    cms = pa_f.tile([64, HS], F32, name="cms")
                    nc.vector.tensor_sub(cms[:], fcos[:], fsin[:])
                    cps = pa_f.tile([64, HS], F32, name="cps")
                    nc.vector.tensor_add(cps[:], fcos[:], fsin[:])
                    nc.sync.dma_start(gk[0:64, fh * HS : (fh + 1) * HS], cms[:])
                    nc.sync.dma_start(
                        gk[64:128, fh * HS : (fh + 1) * HS], cps[:]
                    )
                nc.scalar.mul(gq[:], gk[:], 1.0 / math.sqrt(HD))

            def proj_qk(w_dram, out_dram, g_tile, ev_name, pw):
                """Transposed projection: out[d, s] = w^T.T @ x^T with the
                rotary multiplier applied on evacuation."""
                w_sb = pw.tile([128, KT * DSH], F32R, name="w_sb")
                _w_load(nc, w_sb, w_dram, 0, KT, 8, ("sync", "scalar"))
                with tc.tile_pool(name="qk_ps", bufs=1, space="PSUM") as ps:
                    for sp in range(2):  # seq-chunk pairs
                        psd = [
                            ps.tile([128, 512], F32, name=f"ps{i}")
                            for i in range(8)
                        ]
                        for kc in range(KT):
                            xt = px.tile([128, 1024], F32R, name="xs")
                            eng = nc.sync if kc % 2 == 0 else nc.scalar
                            eng.dma_start(
                                xt[:],
                                xT.ap()[
                                    kc * 128 : (kc + 1) * 128,
                                    sp * 1024 : (sp + 1) * 1024,
                                ],
                            )
                            for half in range(2):
                                for dt in range(HL):
                                    nc.tensor.matmul(
                                        psd[half * HL + dt][:],
                                        w_sb[
                                            :,
                                            kc * DSH
                                            + dt * 128 : kc * DSH
                                            + (dt + 1) * 128,
                                        ],
                                        xt[:, half * 512 : (half + 1) * 512],
                                        start=(kc == 0),
                                        stop=(kc == KT - 1),
                                    )
                        for half in range(2):
                            sc = sp * 2 + half
                            for dt in range(HL):
                                ev = pev.tile(
                                    [128, 512], F32R, name=ev_name
                                )
                                nc.vector.tensor_mul(
                                    ev[:],
                                    psd[half * HL + dt][:],
                                    g_tile[:, sc * 512 : (sc + 1) * 512],
                                )
                                eng = nc.sync if dt % 2 == 0 else nc.scalar
                                eng.dma_start(
                                    out_dram.ap()[
                                        dt * 128 : (dt + 1) * 128,
                                        sc * 512 : (sc + 1) * 512,
                                    ],
                                    ev[:],
                                )

            with tc.tile_pool(name="pw_q", bufs=1) as pw_q:
                proj_qk(wqT, qT_d, gq, "qev", pw_q)
            with tc.tile_pool(name="pw_k", bufs=1) as pw_k:
                proj_qk(wkT, kT_d, gk, "kev", pw_k)

            # K prefetch for phase 2 (kT_d complete now)
            for h in range(HL):
                nc.gpsimd.dma_start(
                    kres[h][:], kT_d.ap()[h * 128 : (h + 1) * 128, :]
                )

            # pass B: V in natural [s, d] layout (x tiles are stationary)
            with (
                tc.tile_pool(name="pw_v", bufs=1) as pw_v,
                tc.tile_pool(name="pb_ps", bufs=1, space="PSUM") as pb_ps,
            ):
                wv_sb = pw_v.tile([128, KT * DSH], F32R)
                _w_load(nc, wv_sb, wvT, 0, KT, 8, ("gpsimd",))
                for sh in range(2):
                    psv = [
                        pb_ps.tile([128, 512], F32, name=f"psv{i}")
                        for i in range(8)
                    ]
                    for kc in range(KT):
                        xt2 = px.tile([128, 1024], F32R, name="xs")
                        eng = nc.sync if kc % 2 == 0 else nc.scalar
                        eng.dma_start(
                            xt2[:],
                            xT.ap()[
                                kc * 128 : (kc + 1) * 128,
                                sh * 1024 : (sh + 1) * 1024,
                            ],
                        )
                        for st in range(8):
                            nc.tensor.matmul(
                                psv[st][:],
                                xt2[:, st * 128 : (st + 1) * 128],
                                wv_sb[:, kc * DSH : (kc + 1) * DSH],
                                start=(kc == 0),
                                stop=(kc == KT - 1),
                            )
                    for st in range(8):
                        gt = sh * 8 + st  # global s-tile 0..15
                        nc.vector.tensor_copy(
                            v_sb[:, gt * DSH : (gt + 1) * DSH], psv[st][:]
                        )

        # ------ phase 2+3: attention, AllGather, output projection ------
        with (
            tc.tile_pool(name="p2_q", bufs=3) as p2_q,
            tc.tile_pool(name="p2_m", bufs=2) as p2_m,
            tc.tile_pool(name="p2_ex", bufs=6) as p2_ex,
            tc.tile_pool(name="p2_sm", bufs=3) as p2_sm,
            tc.tile_pool(name="p2_at", bufs=3) as p2_at,
            tc.tile_pool(name="p3_w", bufs=1) as p3_w,
            tc.tile_pool(name="p3_a", bufs=4) as p3_a,
            tc.tile_pool(name="p3_ev", bufs=4) as p3_ev,
            tc.tile_pool(name="p2_one", bufs=1) as p2_one,
        ):
            ones_t = p2_one.tile([128, 1], F32R)
            nc.sync.dma_start(ones_t[:], ones_in.ap())
            wo_sb = p3_w.tile([128, KT * DSH], F32R)
            _w_load(nc, wo_sb, woT, 0, KT, 8, ("gpsimd",))

            last_attn = {}

            def do_attn(qc, pool_sc, pool_ap, pool_sp):
                live = [kt for kt in range(ST) if cls_grid[kt][qc] != B_SKIP]
                mtiles = {}
                for kt in live:
                    if cls_grid[kt][qc] == B_ADD:
                        mk = p2_m.tile([128, 512], F32, name=f"mk{kt % 4}")
                        nc.scalar.dma_start(
                            mk[:],
                            maskT.ap()[
                                kt * 128 : (kt + 1) * 128,
                                qc * 512 : (qc + 1) * 512,
                            ],
                        )
                        mtiles[kt] = mk
                for h in range(HL):
                    qt = p2_q.tile([128, 512], F32R, name="qt")
                    nc.sync.dma_start(
                        qt[:],
                        qT_d.ap()[
                            h * 128 : (h + 1) * 128, qc * 512 : (qc + 1) * 512
                        ],
                    )
                    att_ps = pool_ap.tile([128, 512], F32, name="att_ps")
                    sum_ps = pool_sp.tile([1, 512], F32, name="sum_ps")
                    for i, kt in enumerate(live):
                        first = i == 0
                        last = i == len(live) - 1
                        sc_ps = pool_sc.tile([128, 512], F32, name="sc_ps")
                        nc.tensor.matmul(
                            sc_ps[:],
                            kres[h][:, kt * 128 : (kt + 1) * 128],
                            qt[:],
                            start=True,
                            stop=True,
                        )
                        if cls_grid[kt][qc] == B_ADD:
                            nc.vector.tensor_add(
                                sc_ps[:], sc_ps[:], mtiles[kt][:]
                            )
                        ex = p2_ex.tile([128, 512], F32R, name="ex")
                        last_attn["scalar"] = nc.scalar.activation(
                            ex[:], sc_ps[:], mybir.ActivationFunctionType.Exp
                        )
                        nc.tensor.matmul(
                            att_ps[:],
                            v_sb[
                                :, kt * DSH + h * 128 : kt * DSH + (h + 1) * 128
                            ],
                            ex[:],
                            start=first,
                            stop=last,
                        )
                        nc.tensor.matmul(
                            sum_ps[:],
                            ones_t[:],
                            ex[:],
                            start=first,
                            stop=last,
                        )
                    rec = p2_sm.tile([1, 512], F32, name="rec")
                    nc.vector.reciprocal_approx_fast(rec[:], sum_ps[0:1, :])
                    rb = p2_sm.tile([128, 512], F32, name="rb")
                    nc.gpsimd.partition_broadcast(rb[:], rec[0:1, :])
                    at = p2_at.tile([128, 512], F32R, name="at")
                    nc.vector.tensor_mul(at[:], att_ps[:], rb[:])
                    last_attn["sync"] = nc.sync.dma_start(
                        attn_sc[qc // 2].ap()[
                            h * 128 : (h + 1) * 128,
                            (qc % 2) * 512 : (qc % 2) * 512 + 512,
                        ],
                        at[:],
                    )

                if qc % 2 == 1:
                    nc.gpsimd.collective_compute(
                        "AllGather",
                        mybir.AluOpType.bypass,
                        ins=[attn_sc[qc // 2].ap()],
                        outs=[attn_full[qc // 2].ap()],
                        replica_groups=[list(range(N_CORES))],
                    )

            def do_p3(qc, pool_ps):
                pso = [
                    pool_ps.tile([128, 512], F32, name=f"pso{i}")
                    for i in range(HL)
                ]
                for dc2 in range(KT // 2):
                    at_t = p3_a.tile([128, 1024], F32R, name="at_t")
                    ename = "sync" if dc2 % 2 == 0 else "scalar"
                    eng = getattr(nc, ename)
                    ld = eng.dma_start(
                        at_t[:].rearrange("p (two s) -> p two s", s=512),
                        attn_full[qc // 2]
                        .ap()[
                            dc2 * 256 : (dc2 + 1) * 256,
                            (qc % 2) * 512 : (qc % 2) * 512 + 512,
                        ]
                        .rearrange("(two p) s -> p two s", p=128),
                    )
                    if dc2 < 2 and ename in last_attn:
                        add_dep_helper(
                            ld.ins,
                            last_attn[ename].ins,
                            sync=False,
                            reason="p3 loads after attention DMAs",
                        )
                    for half in range(2):
                        dc = dc2 * 2 + half
                        for jt in range(HL):
                            nc.tensor.matmul(
                                pso[jt][:],
                                wo_sb[
                                    :,
                                    dc * DSH
                                    + jt * 128 : dc * DSH
                                    + (jt + 1) * 128,
                                ],
                                at_t[:, half * 512 : (half + 1) * 512],
                                start=(dc == 0),
                                stop=(dc == KT - 1),
                            )
                for jt in range(HL):
                    oev = p3_ev.tile([128, 512], F32, name="oev")
                    nc.vector.tensor_copy(oev[:], pso[jt][:])
                    nc.sync.dma_start(
                        outT.ap()[
                            jt * 128 : (jt + 1) * 128, qc * 512 : (qc + 1) * 512
                        ],
                        oev[:],
                    )

            # attention first (deep PE lookahead via 4 score banks), then
            # the output-projection chunks; the AllGathers fly while the PE
            # is still busy with later attention chunks.
            with (
                tc.tile_pool(name="p2_sc", bufs=4, space="PSUM") as psc,
                tc.tile_pool(name="p2_ap", bufs=2, space="PSUM") as pap,
                tc.tile_pool(name="p2_sp", bufs=2, space="PSUM") as psp,
            ):
                for qc in range(SC):
                    do_attn(qc, psc, pap, psp)
            with tc.tile_pool(name="p3_ps", bufs=1, space="PSUM") as pps:
                for qc in range(SC):
                    do_p3(qc, pps)

    nc.compile()
    return nc


def _install_trace_hooks():
    """Install the NTFF profile hook (missing antenv.axon_hooks stub) and
    neutralize the artifact upload so trace=True works in this container."""
    import sys
    import types

    from concourse import bass_utils as _bu

    _bu.upload_artifacts = lambda tmpdir: f"file://{tmpdir}"
    if "antenv.axon_hooks" in sys.modules:
        return
    import antenv

    mod = types.ModuleType("antenv.axon_hooks")
    _h = [None]
    mod.set_axon_ntff_profile_hook = lambda hk: _h.__setitem__(0, hk)
    mod.get_axon_ntff_profile_hook = lambda: _h[0]
    sys.modules["antenv.axon_hooks"] = mod
    antenv.axon_hooks = mod
    from trn_agent_boot.trn_boot import _ntff_profile_via_ctypes

    mod.set_axon_ntff_profile_hook(
        _ntff_profile_via_ctypes("/opt/axon/libaxon_pjrt.so")
    )


_CACHE = {}


def _get_program(cls_grid):
    key = tuple(map(tuple, cls_grid))
    if key not in _CACHE:
        _CACHE[key] = _build(cls_grid)
    return _CACHE[key]


def _classify_mask(maskT_np):
    """Classify each [128k, 512q] block of the transposed mask."""
    grid = []
    for kt in range(ST):
        row = []
        for qc in range(SC):
            blk = maskT_np[kt * 128 : (kt + 1) * 128, qc * 512 : (qc + 1) * 512]
            if np.all(blk < -1e4):
                row.append(B_SKIP)
            elif np.all(blk == 0.0):
                row.append(B_ZERO)
            else:
                row.append(B_ADD)
        grid.append(row)
    return grid


_ONES = np.ones((128, 1), dtype=np.float32)

# within-head permutation: even head_dim indices first, then odd
_PERM = np.empty(DSH, dtype=np.int64)
for _hl in range(HL):
    for _j in range(64):
        _PERM[_hl * 128 + _j] = _hl * 128 + 2 * _j
        _PERM[_hl * 128 + 64 + _j] = _hl * 128 + 2 * _j + 1


def kernel(x, start_pos, freqs, mask, wq, wk, wv, wo):
    x = np.asarray(x, dtype=np.float32)
    freqs = np.asarray(freqs, dtype=np.float32)
    mask = np.asarray(mask, dtype=np.float32)
    wq = np.asarray(wq, dtype=np.float32)
    wk = np.asarray(wk, dtype=np.float32)
    wv = np.asarray(wv, dtype=np.float32)
    wo = np.asarray(wo, dtype=np.float32)

    xs = x.reshape(S, D)
    xT = np.ascontiguousarray(xs.T)
    freqT = np.ascontiguousarray(
        np.concatenate([freqs[:, :, 0].T, freqs[:, :, 1].T], axis=0)
    )  # [128, S]: rows 0-63 cos_j(s), 64-127 sin_j(s)
    maskT_np = np.ascontiguousarray(mask.reshape(S, S).T)
    cls_grid = _classify_mask(maskT_np)
    nc = _get_program(cls_grid)

    in_maps = []
    for c in range(N_CORES):
        rows = slice(c * DSH, (c + 1) * DSH)
        wq_c = wq[rows][_PERM]  # permute within-head rows (even hd, odd hd)
        wk_c = wk[rows][_PERM]
        in_maps.append(
            {
                "xT": xT,
                "wqT": np.ascontiguousarray(wq_c.T),
                "wkT": np.ascontiguousarray(wk_c.T),
                "wvT": np.ascontiguousarray(wv[rows].T),
                "woT": np.ascontiguousarray(wo[rows].T),
                "freqT": freqT,
                "ones_in": _ONES,
                "maskT": maskT_np,
            }
        )

    trace = os.environ.get("ATTN_TRACE") == "1"
    if trace:
        try:
            _install_trace_hooks()
        except Exception:
            pass

    res = run_bass_kernel_spmd(
        nc,
        in_maps,
        list(range(N_CORES)),
        trace=trace,
        trace_cores=[0] if trace else None,
    )
    if trace:
        kernel.last_exec_time_ns = res.exec_time_ns
        kernel.last_results = res

    out = np.empty((S, D), dtype=np.float32)
    for c in range(N_CORES):
        out[:, c * DSH : (c + 1) * DSH] = res.results[c]["outT"].T
    return out[None]
